# revision 23
# baseline (speedup 1.0000x reference)
"""Trainium2 Bass kernel for nn_DigitCapsLayer (dynamic routing capsule layer).

Strategy: shard the 1152-wide input-capsule axis across 8 cores (144 each).
Priors P = x@W are never materialized; each routing iteration runs as bf16
matmuls on the tensor engine (fp32 PSUM accumulate):
  - s_n = sum_il (x^T * c_bcast) W_n      (xc elementwise on DVE, PE contract)
  - c broadcast over l via constant selector matmul E (exact 0/1 in bf16)
  - bb update delta = F^T (x^T * (W_n^T @ s_sum)), squash factor folded into
    the bb accumulate (linear), so G-matmuls start right after the AllReduce
Softmax over batch is along the free dim per (n,i) row; (n,i) rows are packed
4-per-128-partition-tile at 32-aligned strips to satisfy PE tile_position
rules. s partials are AllReduced (160KB fp32) once per routing iteration.

Self-contained: hardcodes shapes from the problem spec.
"""
import os
import sys
import types

import numpy as np

sys.path.insert(0, "/root/.axon_site")
try:  # NTFF profile hook shim (timing only; harmless if unavailable)
    import antenv.axon_hooks  # noqa: F401
except ImportError:
    try:
        from trn_agent_boot import trn_boot as _tb

        _m = types.ModuleType("antenv.axon_hooks")
        _hook = _tb._ntff_profile_via_ctypes("/opt/axon/libaxon_pjrt.so")
        _m.get_axon_ntff_profile_hook = lambda: _hook
        sys.modules["antenv.axon_hooks"] = _m
    except Exception:
        pass

import ml_dtypes

import concourse.bacc as bacc
import concourse.mybir as mybir
import concourse.tile as tile
from concourse import bass_utils

N_CORES = 8
NN = 10       # output capsule classes
B = 256       # batch
I_LOC = 144   # input capsules per core
L = 8         # in capsule dim
O = 16        # out capsule dim
NCHUNK = 9    # 128-row (i,l) chunks per core
NGROUP = NN * NCHUNK          # 90 groups of 16 i's
NTILE = (NGROUP + 3) // 4     # 23 packed bb tiles (4 strips each)
PERM = [0, 4, 1, 5, 2, 6, 3, 7, 8]            # chunk order in xT columns
COL = {ck: i for i, ck in enumerate(PERM)}    # chunk -> xT column block
PAIRS = [(0, 4), (1, 5), (2, 6), (3, 7), (8,)]
F32 = mybir.dt.float32
BF16 = mybir.dt.bfloat16
AF = mybir.ActivationFunctionType
ALU = mybir.AluOpType
BF = ml_dtypes.bfloat16


# ---------------------------------------------------------------- numpy prep
def _constants():
    E_all = np.zeros((128, 128), BF)
    F = np.zeros((128, 32), BF)
    for di in range(16):
        for l in range(L):
            F[di * 8 + l, di] = 1.0
            for j in range(4):
                E_all[32 * j + di, di * 8 + l] = 1.0
    return E_all, F


def _prep_core(x, weight, r):
    i0 = I_LOC * r
    xs = x[:, i0:i0 + I_LOC, :]                       # [B,144,8]
    ws = weight[:, i0:i0 + I_LOC, :, :]               # [10,144,8,16]
    x_il = np.ascontiguousarray(xs.transpose(1, 2, 0).reshape(I_LOC * L, B))
    xT = np.ascontiguousarray(
        x_il.reshape(NCHUNK, 128, B).transpose(1, 0, 2)[:, PERM]
        .reshape(128, NCHUNK * B)
    )
    w_il = ws.reshape(NN, I_LOC * L, O)               # [n,(il),o]
    WA = np.zeros((128, NN * NCHUNK, 32), BF)
    WA[:, :, :O] = w_il.reshape(NN, NCHUNK, 128, O).transpose(2, 0, 1, 3) \
        .reshape(128, NN * NCHUNK, O)
    WA = np.ascontiguousarray(WA.reshape(128, NN * NCHUNK * 32))
    WT = np.ascontiguousarray(w_il.transpose(0, 2, 1).reshape(NN * O, I_LOC * L))
    WT = np.ascontiguousarray(
        WT.reshape(NN, O, I_LOC * L).transpose(1, 0, 2)
        .reshape(O, NN * I_LOC * L).astype(BF)
    )
    return xT, xT.astype(BF), WA, WT


def _in_maps(x, weight):
    E_all, F = _constants()
    maps = []
    for r in range(N_CORES):
        xT, xTb, WA, WT = _prep_core(x, weight, r)
        maps.append({
            "xT": xT, "xTb": xTb, "WA": WA, "WT": WT,
            "EALL": E_all, "FMAT": F,
            "EYE16": np.eye(16, dtype=np.float32),
            "ONES16": np.ones((16, 1), np.float32),
            "ONES1x16": np.ones((1, 16), np.float32),
            "ONES1x128": np.ones((1, 128), np.float32),
        })
    return maps


# ---------------------------------------------------------------- bass build
def build_nc():
    nc = bacc.Bacc(
        "TRN2",
        target_bir_lowering=False,
        debug=False,
        enable_asserts=False,
        num_devices=N_CORES,
    )
    d_xT = nc.dram_tensor("xT", [128, NCHUNK * B], F32, kind="ExternalInput")
    d_xTb = nc.dram_tensor("xTb", [128, NCHUNK * B], BF16, kind="ExternalInput")
    d_WA = nc.dram_tensor("WA", [128, NN * NCHUNK * 32], BF16, kind="ExternalInput")
    d_WT = nc.dram_tensor("WT", [O, NN * I_LOC * L], BF16, kind="ExternalInput")
    d_E = nc.dram_tensor("EALL", [128, 128], BF16, kind="ExternalInput")
    d_F = nc.dram_tensor("FMAT", [128, 32], BF16, kind="ExternalInput")
    d_I16 = nc.dram_tensor("EYE16", [16, 16], F32, kind="ExternalInput")
    d_o16 = nc.dram_tensor("ONES16", [16, 1], F32, kind="ExternalInput")
    d_o1x = nc.dram_tensor("ONES1x16", [1, 16], F32, kind="ExternalInput")
    d_o1y = nc.dram_tensor("ONES1x128", [1, 128], F32, kind="ExternalInput")
    d_out = nc.dram_tensor("v_out", [NN * B, O], F32, kind="ExternalOutput")

    with tile.TileContext(nc) as tc:
        with (
            tc.tile_pool(name="persist", bufs=1) as pp,
            tc.tile_pool(name="work", bufs=10) as wp,
            tc.tile_pool(name="ps_s", bufs=1, space="PSUM") as ps_s,
            tc.tile_pool(name="ps_big", bufs=4, space="PSUM") as ps_big,
            tc.tile_pool(name="ps_delta", bufs=3, space="PSUM") as ps_delta,
            tc.tile_pool(name="dram", bufs=6, space="DRAM") as dp,
        ):
            # ---- persistent SBUF
            xT = pp.tile([128, NCHUNK * B], F32, tag="xT")
            xTb = pp.tile([128, NCHUNK * B], BF16, tag="xTb")
            WA = pp.tile([128, NN * NCHUNK * 32], BF16, tag="WA")
            WT = pp.tile([O, NN * I_LOC * L], BF16, tag="WT")
            EALL = pp.tile([128, 128], BF16, tag="EALL")
            FMAT = pp.tile([128, 32], BF16, tag="FMAT")
            EYE16 = pp.tile([16, 16], F32, tag="EYE16")
            ONES16 = pp.tile([16, 1], F32, tag="ONES16")
            ONES1x16 = pp.tile([1, 16], F32, tag="ONES1x16")
            ONES1x128 = pp.tile([1, 128], F32, tag="ONES1x128")
            bb = pp.tile([128, NTILE * B], F32, tag="bb")
            expb = pp.tile([128, NTILE * B], F32, tag="expb")
            csb = pp.tile([128, NTILE * B], BF16, tag="csb")
            den = pp.tile([128, NTILE], F32, tag="den")
            denr = pp.tile([128, NTILE], F32, tag="denr")
            s_stage4 = pp.tile([128, 3 * B], F32, tag="s_stage4")
            ssum = pp.tile([O, NN * B], F32, tag="ssum")
            ssb = pp.tile([O, NN * B], BF16, tag="ssb")
            sq_scr = pp.tile([O, NN * B], F32, tag="sq_scr")
            vsb = pp.tile([O, NN * B], F32, tag="vsb")
            vout = pp.tile([128, 20 * O], F32, tag="vout")
            q16 = pp.tile([O, 1], F32, tag="q16")
            sc_r = pp.tile([1, 1], F32, tag="sc_r")
            sc_d = pp.tile([1, 1], F32, tag="sc_d")
            sc_dr = pp.tile([1, 1], F32, tag="sc_dr")
            sc_f = pp.tile([1, 1], F32, tag="sc_f")
            f16 = pp.tile([O, 1], F32, tag="f16")
            f128 = pp.tile([128, 1], F32, tag="f128")

            # ---- load inputs
            nc.sync.dma_start(xTb[:], d_xTb.ap())
            for n_ in range(NN):
                w0, w1 = n_ * NCHUNK * 32, (n_ + 1) * NCHUNK * 32
                nc.sync.dma_start(WA[:, w0:w1], d_WA.ap()[:, w0:w1])
            nc.sync.dma_start(xT[:], d_xT.ap())
            nc.sync.dma_start(WT[:], d_WT.ap())
            nc.sync.dma_start(EALL[:], d_E.ap())
            nc.sync.dma_start(FMAT[:], d_F.ap())
            nc.sync.dma_start(EYE16[:], d_I16.ap())
            nc.sync.dma_start(ONES16[:], d_o16.ap())
            nc.sync.dma_start(ONES1x16[:], d_o1x.ap())
            nc.sync.dma_start(ONES1x128[:], d_o1y.ap())
            nc.gpsimd.memset(bb[:], 0.0)

            cc_in = [
                dp.tile([O, NN * B], F32, tag=f"cc_in{k}", name=f"cc_in{k}")
                for k in range(3)
            ]
            cc_out = [
                dp.tile([O, NN * B], F32, tag=f"cc_out{k}", name=f"cc_out{k}")
                for k in range(3)
            ]

            def ck_pairs():
                return [(0, 2), (2, 4), (4, 6), (6, 8), (8, 9)]

            def s_pass(k, rhs_of, n0=None):
                """s-matmuls packed 4 classes per PSUM tile at col strips
                (stationary padded to M=32 so strips are fully written);
                drain via SBUF staging, then partition-shifting DMAs into
                the collective DRAM buffer."""
                packs = range(0, NN, 4) if n0 is None else [n0]
                for n0 in packs:
                    nhi = min(n0 + 4, NN)
                    pk = n0 // 4
                    s4 = ps_s.tile([128, B], F32, tag="s_acc", name=f"s4_{k}_{n0}")
                    for n in range(n0, nhi):
                        jn = n - n0
                        for ck in range(NCHUNK):
                            g = n * NCHUNK + ck
                            nc.tensor.matmul(
                                s4[32 * jn:32 * jn + 32, :],
                                lhsT=WA[:, g * 32:(g + 1) * 32],
                                rhs=rhs_of(n, ck),
                                start=(ck == 0),
                                stop=(ck == NCHUNK - 1),
                                tile_position=(0, 32 * jn),
                            )
                    p_hi = 32 * (nhi - n0)
                    nc.scalar.activation(
                        s_stage4[:p_hi, pk * B:(pk + 1) * B], s4[:p_hi, :], AF.Copy
                    )
                    for n in range(n0, nhi):
                        jn = n - n0
                        nc.sync.dma_start(
                            cc_in[k][:, n * B:(n + 1) * B],
                            s_stage4[32 * jn:32 * jn + 16, pk * B:(pk + 1) * B],
                        )

            def allreduce(k, alpha, last):
                nc.gpsimd.collective_compute(
                    "AllReduce",
                    ALU.add,
                    replica_groups=[list(range(N_CORES))],
                    ins=[cc_in[k].opt()],
                    outs=[cc_out[k].opt()],
                )
                nc.sync.dma_start(ssum[:], cc_out[k][:])
                if not last:
                    for n_ in range(NN):
                        nc.scalar.activation(
                            ssb[:, n_ * B:(n_ + 1) * B],
                            ssum[:, n_ * B:(n_ + 1) * B], AF.Copy,
                        )
                # squash scalar chain -> factor (applied later at bb-add / v)
                nc.scalar.activation(sq_scr[:], ssum[:], AF.Square, accum_out=q16[:])
                n2_ps = ps_delta.tile([1, 1], F32, tag="delta", name=f"n2_{k}")
                nc.tensor.matmul(n2_ps[:], lhsT=ONES16[:], rhs=q16[:])
                a2 = float(alpha * alpha)
                nc.scalar.activation(sc_r[:], n2_ps[:], AF.Sqrt, scale=a2)
                nc.scalar.activation(sc_d[:], n2_ps[:], AF.Copy, bias=1.0, scale=a2)
                nc.vector.reciprocal(sc_dr[:], sc_d[:])
                nc.vector.scalar_tensor_tensor(
                    out=sc_f[:], in0=sc_r[:], scalar=float(alpha), in1=sc_dr[:],
                    op0=ALU.mult, op1=ALU.mult,
                )
                if last:
                    f16_ps = ps_delta.tile([O, 1], F32, tag="delta", name=f"f16_{k}")
                    nc.tensor.matmul(f16_ps[:], lhsT=ONES1x16[:], rhs=sc_f[:])
                    nc.vector.tensor_copy(f16[:], f16_ps[:])
                    for n in range(NN):
                        nc.scalar.activation(
                            vsb[:, n * B:(n + 1) * B], ssum[:, n * B:(n + 1) * B],
                            AF.Copy, scale=f16[:],
                        )
                else:
                    f128_ps = ps_delta.tile([128, 1], F32, tag="delta", name=f"f128_{k}")
                    nc.tensor.matmul(f128_ps[:], lhsT=ONES1x128[:], rhs=sc_f[:])
                    nc.vector.tensor_copy(f128[:], f128_ps[:])

            # ================= phase 1: uniform-c s1 =================
            with nc.named_scope("phase_s1"):
                s_pass(0, lambda n, ck: xTb[:, COL[ck] * B:(COL[ck] + 1) * B])
            with nc.named_scope("ar1"):
                allreduce(0, 1.0 / B, last=False)

            # ================= routing phases 2,3 =================
            for it in (1, 2):
                with nc.named_scope(f"bbupd{it}"):
                    # G'-mms on un-squashed s_sum; factor folded into bb-add
                    deltas = {}

                    def fmm(g, rhs_ap, it=it):
                        t, j = g // 4, g % 4
                        if t not in deltas:
                            deltas[t] = [
                                ps_delta.tile([128, B], F32, tag="delta",
                                              name=f"delta_{it}_{t}"),
                                0,
                            ]
                        ent = deltas[t]
                        nc.tensor.matmul(
                            ent[0][32 * j:32 * j + 32, :],
                            lhsT=FMAT[:], rhs=rhs_ap,
                            tile_position=(0, 32 * j),
                        )
                        ent[1] += 1
                        full = 2 if t == NTILE - 1 else 4
                        if ent[1] == full:
                            p_hi = 32 * full
                            nc.vector.scalar_tensor_tensor(
                                out=bb[:p_hi, t * B:(t + 1) * B],
                                in0=ent[0][:p_hi, :],
                                scalar=f128[:p_hi, 0:1],
                                in1=bb[:p_hi, t * B:(t + 1) * B],
                                op0=ALU.mult, op1=ALU.add,
                            )
                            del deltas[t]

                    xT9 = xT[:].rearrange("p (c b) -> p c b", c=NCHUNK)
                    for n in range(NN):
                        tmps = {}
                        for (c0, c1) in ck_pairs():
                            w = (c1 - c0) * B
                            G = ps_big.tile([128, 2 * B], F32, tag="big",
                                            name=f"G_{it}_{n}_{c0}")
                            for ck in range(c0, c1):
                                h = ck - c0
                                nc.tensor.matmul(
                                    G[:, h * B:(h + 1) * B],
                                    lhsT=WT[:, n * 1152 + ck * 128: n * 1152 + (ck + 1) * 128],
                                    rhs=ssb[:, n * B:(n + 1) * B],
                                )
                            tmp = wp.tile([128, 2 * B], BF16, tag="tmp")
                            if c1 - c0 == 2:
                                in0 = xT9[:, COL[c0]:COL[c0] + 3:2, :]
                                nc.vector.tensor_mul(
                                    tmp[:].rearrange("p (c b) -> p c b", c=2),
                                    in0,
                                    G[:].rearrange("p (c b) -> p c b", c=2),
                                )
                            else:
                                nc.vector.tensor_mul(
                                    tmp[:, :w], xT[:, COL[c0] * B:(COL[c0] + 1) * B],
                                    G[:, :w],
                                )
                            for ck in range(c0, c1):
                                tmps[ck] = (tmp, ck - c0)
                        for ck in range(NCHUNK):
                            tmp, h = tmps[ck]
                            fmm(n * NCHUNK + ck, tmp[:, h * B:(h + 1) * B])
                with nc.named_scope(f"softmax{it}"):
                    for t in range(NTILE):
                        nc.scalar.activation(
                            expb[:, t * B:(t + 1) * B], bb[:, t * B:(t + 1) * B],
                            AF.Exp, accum_out=den[:, t:t + 1],
                        )
                        nc.vector.reciprocal(denr[:, t:t + 1], den[:, t:t + 1])
                        nc.gpsimd.tensor_scalar_mul(
                            csb[:, t * B:(t + 1) * B], expb[:, t * B:(t + 1) * B],
                            denr[:, t:t + 1],
                        )
                with nc.named_scope(f"schain{it}"):
                    xc_of = {}

                    def emit_exc(n, it=it):
                        for pi, pair in enumerate(PAIRS):
                            w = len(pair) * B
                            g0 = n * NCHUNK + pair[0]
                            t, j = g0 // 4, g0 % 4
                            ct = ps_big.tile([128, 2 * B], F32, tag="big",
                                             name=f"ct_{it}_{n}_{pi}")
                            nc.tensor.matmul(
                                ct[:, :w],
                                lhsT=EALL[32 * j:32 * j + 16, :],
                                rhs=csb[32 * j:32 * j + 16, t * B:t * B + w],
                                tile_position=(32 * j, 0),
                            )
                            xc = wp.tile([128, 2 * B], BF16, tag="xc")
                            nc.vector.tensor_mul(
                                xc[:, :w],
                                xT[:, 2 * pi * B:2 * pi * B + w], ct[:, :w]
                            )
                            for h, ck in enumerate(pair):
                                xc_of[(n, ck)] = (xc, h)

                    def rhs_of(n, ck):
                        xc, h = xc_of[(n, ck)]
                        return xc[:, h * B:(h + 1) * B]

                    for n0 in range(0, NN, 4):
                        for n in range(n0, min(n0 + 4, NN)):
                            emit_exc(n)
                        s_pass(it, rhs_of, n0=n0)
                with nc.named_scope(f"ar{it + 1}"):
                    allreduce(it, 1.0, last=(it == 2))

            # ================= output =================
            with nc.named_scope("out"):
                for k in range(20):
                    vt = ps_delta.tile([128, O], F32, tag="delta", name=f"vt_{k}")
                    nc.tensor.transpose(
                        vt[:], vsb[:, k * 128:(k + 1) * 128], EYE16[:]
                    )
                    nc.scalar.activation(vout[:, k * O:(k + 1) * O], vt[:], AF.Copy)
                nc.sync.dma_start(
                    d_out.ap().rearrange("(k p) o -> p k o", p=128),
                    vout[:].rearrange("p (k o) -> p k o", k=20),
                )
    nc.compile()
    return nc


_NC = None


def _get_nc():
    global _NC
    if _NC is None:
        _NC = build_nc()
    return _NC


def run_spmd(x, weight, trace=False, **kw):
    nc = _get_nc()
    res = bass_utils.run_bass_kernel_spmd(
        nc, _in_maps(np.asarray(x), np.asarray(weight)),
        core_ids=list(range(N_CORES)), trace=trace, **kw,
    )
    return res


def kernel(x, weight):
    res = run_spmd(x, weight, trace=False)
    v = res.results[0]["v_out"]                    # [2560, 16]
    return v.reshape(NN, B, 1, 1, O).astype(np.float32)


# revision 24
# speedup vs baseline: 1.2822x; 1.2822x over previous
"""Trainium2 Bass kernel for nn_DigitCapsLayer (dynamic routing capsule layer).

Strategy: shard the 1152-wide input-capsule axis across 8 cores (144 each).
Priors P = x@W are never materialized; each routing iteration runs as bf16
matmuls on the tensor engine (fp32 PSUM accumulate):
  - s_n = sum_il (x^T * c_bcast) W_n      (xc elementwise on DVE, PE contract)
  - c broadcast over l via constant selector matmul E (exact 0/1 in bf16)
  - bb update delta = F^T (x^T * (W_n^T @ s_sum)), squash factor folded into
    the bb accumulate (linear), so G-matmuls start right after the AllReduce
Softmax over batch is along the free dim per (n,i) row; (n,i) rows are packed
4-per-128-partition-tile at 32-aligned strips to satisfy PE tile_position
rules. s partials are AllReduced (160KB fp32) once per routing iteration.

Self-contained: hardcodes shapes from the problem spec.
"""
import os
import sys
import types

import numpy as np

sys.path.insert(0, "/root/.axon_site")
try:  # NTFF profile hook shim (timing only; harmless if unavailable)
    import antenv.axon_hooks  # noqa: F401
except ImportError:
    try:
        from trn_agent_boot import trn_boot as _tb

        _m = types.ModuleType("antenv.axon_hooks")
        _hook = _tb._ntff_profile_via_ctypes("/opt/axon/libaxon_pjrt.so")
        _m.get_axon_ntff_profile_hook = lambda: _hook
        sys.modules["antenv.axon_hooks"] = _m
    except Exception:
        pass

import ml_dtypes

import concourse.bacc as bacc
import concourse.mybir as mybir
import concourse.tile as tile
from concourse import bass_utils

N_CORES = 8
NN = 10       # output capsule classes
B = 256       # batch
I_LOC = 144   # input capsules per core
L = 8         # in capsule dim
O = 16        # out capsule dim
NCHUNK = 9    # 128-row (i,l) chunks per core
NGROUP = NN * NCHUNK          # 90 groups of 16 i's
NTILE = (NGROUP + 3) // 4     # 23 packed bb tiles (4 strips each)
PERM = [0, 4, 1, 5, 2, 6, 3, 7, 8]            # chunk order in xT columns
COL = {ck: i for i, ck in enumerate(PERM)}    # chunk -> xT column block
PAIRS = [(0, 4), (1, 5), (2, 6), (3, 7), (8,)]
F32 = mybir.dt.float32
BF16 = mybir.dt.bfloat16
AF = mybir.ActivationFunctionType
ALU = mybir.AluOpType
BF = ml_dtypes.bfloat16


# ---------------------------------------------------------------- numpy prep
def _constants():
    E_all = np.zeros((128, 128), BF)
    F = np.zeros((128, 32), BF)
    for di in range(16):
        for l in range(L):
            F[di * 8 + l, di] = 1.0
            for j in range(4):
                E_all[32 * j + di, di * 8 + l] = 1.0
    return E_all, F


def _prep_core(x, weight, r):
    i0 = I_LOC * r
    xs = x[:, i0:i0 + I_LOC, :]                       # [B,144,8]
    ws = weight[:, i0:i0 + I_LOC, :, :]               # [10,144,8,16]
    x_il = np.ascontiguousarray(xs.transpose(1, 2, 0).reshape(I_LOC * L, B))
    xT = np.ascontiguousarray(
        x_il.reshape(NCHUNK, 128, B).transpose(1, 0, 2)[:, PERM]
        .reshape(128, NCHUNK * B)
    )
    w_il = ws.reshape(NN, I_LOC * L, O)               # [n,(il),o]
    WA = np.zeros((128, NN * NCHUNK, 32), BF)
    WA[:, :, :O] = w_il.reshape(NN, NCHUNK, 128, O).transpose(2, 0, 1, 3) \
        .reshape(128, NN * NCHUNK, O)
    WA = np.ascontiguousarray(WA.reshape(128, NN * NCHUNK * 32))
    WT = np.ascontiguousarray(w_il.transpose(0, 2, 1).reshape(NN * O, I_LOC * L))
    WT = np.ascontiguousarray(
        WT.reshape(NN, O, I_LOC * L).transpose(1, 0, 2)
        .reshape(O, NN * I_LOC * L).astype(BF)
    )
    return xT, xT.astype(BF), WA, WT


def _in_maps(x, weight):
    E_all, F = _constants()
    maps = []
    for r in range(N_CORES):
        xT, xTb, WA, WT = _prep_core(x, weight, r)
        maps.append({
            "xT": xT, "xTb": xTb, "WA": WA, "WT": WT,
            "EALL": E_all, "FMAT": F,
            "EYE16": np.eye(16, dtype=np.float32),
            "ONES16": np.ones((16, 1), np.float32),
            "ONES1x16": np.ones((1, 16), np.float32),
            "ONES1x128": np.ones((1, 128), np.float32),
        })
    return maps


# ---------------------------------------------------------------- bass build
def build_nc():
    nc = bacc.Bacc(
        "TRN2",
        target_bir_lowering=False,
        debug=False,
        enable_asserts=False,
        num_devices=N_CORES,
    )
    d_xT = nc.dram_tensor("xT", [128, NCHUNK * B], F32, kind="ExternalInput")
    d_xTb = nc.dram_tensor("xTb", [128, NCHUNK * B], BF16, kind="ExternalInput")
    d_WA = nc.dram_tensor("WA", [128, NN * NCHUNK * 32], BF16, kind="ExternalInput")
    d_WT = nc.dram_tensor("WT", [O, NN * I_LOC * L], BF16, kind="ExternalInput")
    d_E = nc.dram_tensor("EALL", [128, 128], BF16, kind="ExternalInput")
    d_F = nc.dram_tensor("FMAT", [128, 32], BF16, kind="ExternalInput")
    d_I16 = nc.dram_tensor("EYE16", [16, 16], F32, kind="ExternalInput")
    d_o16 = nc.dram_tensor("ONES16", [16, 1], F32, kind="ExternalInput")
    d_o1x = nc.dram_tensor("ONES1x16", [1, 16], F32, kind="ExternalInput")
    d_o1y = nc.dram_tensor("ONES1x128", [1, 128], F32, kind="ExternalInput")
    d_out = nc.dram_tensor("v_out", [NN * B, O], F32, kind="ExternalOutput")

    with tile.TileContext(nc) as tc:
        with (
            tc.tile_pool(name="persist", bufs=1) as pp,
            tc.tile_pool(name="work", bufs=10) as wp,
            tc.tile_pool(name="ps_s", bufs=1, space="PSUM") as ps_s,
            tc.tile_pool(name="ps_big", bufs=4, space="PSUM") as ps_big,
            tc.tile_pool(name="ps_delta", bufs=3, space="PSUM") as ps_delta,
            tc.tile_pool(name="dram", bufs=6, space="DRAM") as dp,
        ):
            # ---- persistent SBUF
            xT = pp.tile([128, NCHUNK * B], F32, tag="xT")
            xTb = pp.tile([128, NCHUNK * B], BF16, tag="xTb")
            WA = pp.tile([128, NN * NCHUNK * 32], BF16, tag="WA")
            WT = pp.tile([O, NN * I_LOC * L], BF16, tag="WT")
            EALL = pp.tile([128, 128], BF16, tag="EALL")
            FMAT = pp.tile([128, 32], BF16, tag="FMAT")
            EYE16 = pp.tile([16, 16], F32, tag="EYE16")
            ONES16 = pp.tile([16, 1], F32, tag="ONES16")
            ONES1x16 = pp.tile([1, 16], F32, tag="ONES1x16")
            ONES1x128 = pp.tile([1, 128], F32, tag="ONES1x128")
            bb = pp.tile([128, NTILE * B], F32, tag="bb")
            expb = pp.tile([128, NTILE * B], F32, tag="expb")
            csb = pp.tile([128, NTILE * B], BF16, tag="csb")
            den = pp.tile([128, NTILE], F32, tag="den")
            denr = pp.tile([128, NTILE], F32, tag="denr")
            s_stage4 = pp.tile([128, 3 * B], F32, tag="s_stage4")
            ssum = pp.tile([O, NN * B], F32, tag="ssum")
            ssb = pp.tile([O, NN * B], BF16, tag="ssb")
            sq_scr = pp.tile([O, NN * B], F32, tag="sq_scr")
            vsb = pp.tile([O, NN * B], F32, tag="vsb")
            vout = pp.tile([128, 20 * O], F32, tag="vout")
            q16 = pp.tile([O, 1], F32, tag="q16")
            sc_r = pp.tile([1, 1], F32, tag="sc_r")
            sc_d = pp.tile([1, 1], F32, tag="sc_d")
            sc_dr = pp.tile([1, 1], F32, tag="sc_dr")
            sc_f = pp.tile([1, 1], F32, tag="sc_f")
            f16 = pp.tile([O, 1], F32, tag="f16")
            f128 = pp.tile([128, 1], F32, tag="f128")

            # ---- load inputs
            nc.sync.dma_start(xTb[:], d_xTb.ap())
            for n_ in range(NN):
                w0, w1 = n_ * NCHUNK * 32, (n_ + 1) * NCHUNK * 32
                nc.sync.dma_start(WA[:, w0:w1], d_WA.ap()[:, w0:w1])
            nc.sync.dma_start(xT[:], d_xT.ap())
            nc.sync.dma_start(WT[:], d_WT.ap())
            nc.sync.dma_start(EALL[:], d_E.ap())
            nc.sync.dma_start(FMAT[:], d_F.ap())
            nc.sync.dma_start(EYE16[:], d_I16.ap())
            nc.sync.dma_start(ONES16[:], d_o16.ap())
            nc.sync.dma_start(ONES1x16[:], d_o1x.ap())
            nc.sync.dma_start(ONES1x128[:], d_o1y.ap())
            nc.gpsimd.memset(bb[:], 0.0)

            cc_in = [
                dp.tile([O, NN * B], F32, tag=f"cc_in{k}", name=f"cc_in{k}")
                for k in range(3)
            ]
            cc_out = [
                dp.tile([O, NN * B], F32, tag=f"cc_out{k}", name=f"cc_out{k}")
                for k in range(3)
            ]

            def ck_pairs():
                return [(0, 2), (2, 4), (4, 6), (6, 8), (8, 9)]

            def s_pass(k, rhs_of, n0=None):
                """s-matmuls packed 4 classes per PSUM tile at col strips
                (stationary padded to M=32 so strips are fully written);
                drain via SBUF staging, then partition-shifting DMAs into
                the collective DRAM buffer."""
                packs = range(0, NN, 4) if n0 is None else [n0]
                for n0 in packs:
                    nhi = min(n0 + 4, NN)
                    pk = n0 // 4
                    s4 = ps_s.tile([128, B], F32, tag="s_acc", name=f"s4_{k}_{n0}")
                    for n in range(n0, nhi):
                        jn = n - n0
                        for ck in range(NCHUNK):
                            g = n * NCHUNK + ck
                            nc.tensor.matmul(
                                s4[32 * jn:32 * jn + 32, :],
                                lhsT=WA[:, g * 32:(g + 1) * 32],
                                rhs=rhs_of(n, ck),
                                start=(ck == 0),
                                stop=(ck == NCHUNK - 1),
                                tile_position=(0, 32 * jn),
                            )
                    p_hi = 32 * (nhi - n0)
                    nc.scalar.activation(
                        s_stage4[:p_hi, pk * B:(pk + 1) * B], s4[:p_hi, :], AF.Copy
                    )
                    for n in range(n0, nhi):
                        jn = n - n0
                        nc.sync.dma_start(
                            cc_in[k][:, n * B:(n + 1) * B],
                            s_stage4[32 * jn:32 * jn + 16, pk * B:(pk + 1) * B],
                        )

            def allreduce(k, alpha, last):
                nc.gpsimd.collective_compute(
                    "AllReduce",
                    ALU.add,
                    replica_groups=[list(range(N_CORES))],
                    ins=[cc_in[k].opt()],
                    outs=[cc_out[k].opt()],
                )
                nc.sync.dma_start(ssum[:], cc_out[k][:])
                if not last:
                    for n_ in range(NN):
                        nc.scalar.activation(
                            ssb[:, n_ * B:(n_ + 1) * B],
                            ssum[:, n_ * B:(n_ + 1) * B], AF.Copy,
                        )
                # squash scalar chain -> factor (applied later at bb-add / v)
                nc.scalar.activation(sq_scr[:], ssum[:], AF.Square, accum_out=q16[:])
                n2_ps = ps_delta.tile([1, 1], F32, tag="delta", name=f"n2_{k}")
                nc.tensor.matmul(n2_ps[:], lhsT=ONES16[:], rhs=q16[:])
                a2 = float(alpha * alpha)
                nc.scalar.activation(sc_r[:], n2_ps[:], AF.Sqrt, scale=a2)
                nc.scalar.activation(sc_d[:], n2_ps[:], AF.Copy, bias=1.0, scale=a2)
                nc.vector.reciprocal(sc_dr[:], sc_d[:])
                nc.vector.scalar_tensor_tensor(
                    out=sc_f[:], in0=sc_r[:], scalar=float(alpha), in1=sc_dr[:],
                    op0=ALU.mult, op1=ALU.mult,
                )
                if last:
                    f16_ps = ps_delta.tile([O, 1], F32, tag="delta", name=f"f16_{k}")
                    nc.tensor.matmul(f16_ps[:], lhsT=ONES1x16[:], rhs=sc_f[:])
                    nc.vector.tensor_copy(f16[:], f16_ps[:])
                    for n in range(NN):
                        nc.scalar.activation(
                            vsb[:, n * B:(n + 1) * B], ssum[:, n * B:(n + 1) * B],
                            AF.Copy, scale=f16[:],
                        )
                else:
                    f128_ps = ps_delta.tile([128, 1], F32, tag="delta", name=f"f128_{k}")
                    nc.tensor.matmul(f128_ps[:], lhsT=ONES1x128[:], rhs=sc_f[:])
                    nc.vector.tensor_copy(f128[:], f128_ps[:])

            # ================= phase 1: uniform-c s1 =================
            with nc.named_scope("phase_s1"):
                s_pass(0, lambda n, ck: xTb[:, COL[ck] * B:(COL[ck] + 1) * B])
            with nc.named_scope("ar1"):
                allreduce(0, 1.0 / B, last=False)

            # ================= routing phases 2,3 =================
            for it in (1, 2):
                with nc.named_scope(f"bbupd{it}"):
                    # G'-mms on un-squashed s_sum; factor folded into bb-add
                    deltas = {}

                    def fmm(g, rhs_ap, it=it):
                        t, j = g // 4, g % 4
                        if t not in deltas:
                            deltas[t] = [
                                ps_delta.tile([128, B], F32, tag="delta",
                                              name=f"delta_{it}_{t}"),
                                0,
                            ]
                        ent = deltas[t]
                        nc.tensor.matmul(
                            ent[0][32 * j:32 * j + 32, :],
                            lhsT=FMAT[:], rhs=rhs_ap,
                            tile_position=(0, 32 * j),
                        )
                        ent[1] += 1
                        full = 2 if t == NTILE - 1 else 4
                        if ent[1] == full:
                            p_hi = 32 * full
                            nc.vector.scalar_tensor_tensor(
                                out=bb[:p_hi, t * B:(t + 1) * B],
                                in0=ent[0][:p_hi, :],
                                scalar=f128[:p_hi, 0:1],
                                in1=bb[:p_hi, t * B:(t + 1) * B],
                                op0=ALU.mult, op1=ALU.add,
                            )
                            del deltas[t]

                    xT9 = xT[:].rearrange("p (c b) -> p c b", c=NCHUNK)
                    for n in range(NN):
                        tmps = {}
                        for (c0, c1) in ck_pairs():
                            w = (c1 - c0) * B
                            G = ps_big.tile([128, 2 * B], F32, tag="big",
                                            name=f"G_{it}_{n}_{c0}")
                            for ck in range(c0, c1):
                                h = ck - c0
                                nc.tensor.matmul(
                                    G[:, h * B:(h + 1) * B],
                                    lhsT=WT[:, n * 1152 + ck * 128: n * 1152 + (ck + 1) * 128],
                                    rhs=ssb[:, n * B:(n + 1) * B],
                                )
                            tmp = wp.tile([128, 2 * B], BF16, tag="tmp")
                            if c1 - c0 == 2:
                                in0 = xT9[:, COL[c0]:COL[c0] + 3:2, :]
                                nc.vector.tensor_mul(
                                    tmp[:].rearrange("p (c b) -> p c b", c=2),
                                    in0,
                                    G[:].rearrange("p (c b) -> p c b", c=2),
                                )
                            else:
                                nc.vector.tensor_mul(
                                    tmp[:, :w], xT[:, COL[c0] * B:(COL[c0] + 1) * B],
                                    G[:, :w],
                                )
                            for ck in range(c0, c1):
                                tmps[ck] = (tmp, ck - c0)
                        for ck in range(NCHUNK):
                            tmp, h = tmps[ck]
                            fmm(n * NCHUNK + ck, tmp[:, h * B:(h + 1) * B])
                with nc.named_scope(f"softmax{it}"):
                    for t in range(NTILE):
                        nc.scalar.activation(
                            expb[:, t * B:(t + 1) * B], bb[:, t * B:(t + 1) * B],
                            AF.Exp, accum_out=den[:, t:t + 1],
                        )
                        nc.vector.reciprocal(denr[:, t:t + 1], den[:, t:t + 1])
                    for t in range(NTILE):
                        nc.scalar.activation(
                            csb[:, t * B:(t + 1) * B], expb[:, t * B:(t + 1) * B],
                            AF.Copy, scale=denr[:, t:t + 1],
                        )
                with nc.named_scope(f"schain{it}"):
                    xc_of = {}

                    def emit_exc(n, it=it):
                        for pi, pair in enumerate(PAIRS):
                            w = len(pair) * B
                            g0 = n * NCHUNK + pair[0]
                            t, j = g0 // 4, g0 % 4
                            ct = ps_big.tile([128, 2 * B], F32, tag="big",
                                             name=f"ct_{it}_{n}_{pi}")
                            nc.tensor.matmul(
                                ct[:, :w],
                                lhsT=EALL[32 * j:32 * j + 16, :],
                                rhs=csb[32 * j:32 * j + 16, t * B:t * B + w],
                                tile_position=(32 * j, 0),
                            )
                            xc = wp.tile([128, 2 * B], BF16, tag="xc")
                            nc.vector.tensor_mul(
                                xc[:, :w],
                                xT[:, 2 * pi * B:2 * pi * B + w], ct[:, :w]
                            )
                            for h, ck in enumerate(pair):
                                xc_of[(n, ck)] = (xc, h)

                    def rhs_of(n, ck):
                        xc, h = xc_of[(n, ck)]
                        return xc[:, h * B:(h + 1) * B]

                    for n0 in range(0, NN, 4):
                        for n in range(n0, min(n0 + 4, NN)):
                            emit_exc(n)
                        s_pass(it, rhs_of, n0=n0)
                with nc.named_scope(f"ar{it + 1}"):
                    allreduce(it, 1.0, last=(it == 2))

            # ================= output =================
            with nc.named_scope("out"):
                for k in range(20):
                    vt = ps_delta.tile([128, O], F32, tag="delta", name=f"vt_{k}")
                    nc.tensor.transpose(
                        vt[:], vsb[:, k * 128:(k + 1) * 128], EYE16[:]
                    )
                    nc.scalar.activation(vout[:, k * O:(k + 1) * O], vt[:], AF.Copy)
                nc.sync.dma_start(
                    d_out.ap().rearrange("(k p) o -> p k o", p=128),
                    vout[:].rearrange("p (k o) -> p k o", k=20),
                )
    nc.compile()
    return nc


_NC = None


def _get_nc():
    global _NC
    if _NC is None:
        _NC = build_nc()
    return _NC


def run_spmd(x, weight, trace=False, **kw):
    nc = _get_nc()
    res = bass_utils.run_bass_kernel_spmd(
        nc, _in_maps(np.asarray(x), np.asarray(weight)),
        core_ids=list(range(N_CORES)), trace=trace, **kw,
    )
    return res


def kernel(x, weight):
    res = run_spmd(x, weight, trace=False)
    v = res.results[0]["v_out"]                    # [2560, 16]
    return v.reshape(NN, B, 1, 1, O).astype(np.float32)


# revision 25
# speedup vs baseline: 1.3341x; 1.0405x over previous
"""Trainium2 Bass kernel for nn_DigitCapsLayer (dynamic routing capsule layer).

Strategy: shard the 1152-wide input-capsule axis across 8 cores (144 each).
Priors P = x@W are never materialized; each routing iteration runs as bf16
matmuls on the tensor engine (fp32 PSUM accumulate):
  - s_n = sum_il (x^T * c_bcast) W_n      (xc elementwise on DVE, PE contract)
  - c broadcast over l via constant selector matmul E (exact 0/1 in bf16)
  - bb update delta = F^T (x^T * (W_n^T @ s_sum)), squash factor folded into
    the bb accumulate (linear), so G-matmuls start right after the AllReduce
Softmax over batch is along the free dim per (n,i) row; (n,i) rows are packed
4-per-128-partition-tile at 32-aligned strips to satisfy PE tile_position
rules. s partials are AllReduced (160KB fp32) once per routing iteration.

Self-contained: hardcodes shapes from the problem spec.
"""
import os
import sys
import types

import numpy as np

sys.path.insert(0, "/root/.axon_site")
try:  # NTFF profile hook shim (timing only; harmless if unavailable)
    import antenv.axon_hooks  # noqa: F401
except ImportError:
    try:
        from trn_agent_boot import trn_boot as _tb

        _m = types.ModuleType("antenv.axon_hooks")
        _hook = _tb._ntff_profile_via_ctypes("/opt/axon/libaxon_pjrt.so")
        _m.get_axon_ntff_profile_hook = lambda: _hook
        sys.modules["antenv.axon_hooks"] = _m
    except Exception:
        pass

import ml_dtypes

import concourse.bacc as bacc
import concourse.mybir as mybir
import concourse.tile as tile
from concourse import bass_utils

N_CORES = 8
NN = 10       # output capsule classes
B = 256       # batch
I_LOC = 144   # input capsules per core
L = 8         # in capsule dim
O = 16        # out capsule dim
NCHUNK = 9    # 128-row (i,l) chunks per core
NGROUP = NN * NCHUNK          # 90 groups of 16 i's
NTILE = (NGROUP + 3) // 4     # 23 packed bb tiles (4 strips each)
PERM = [0, 4, 1, 5, 2, 6, 3, 7, 8]            # chunk order in xT columns
COL = {ck: i for i, ck in enumerate(PERM)}    # chunk -> xT column block
PAIRS = [(0, 4), (1, 5), (2, 6), (3, 7), (8,)]
F32 = mybir.dt.float32
BF16 = mybir.dt.bfloat16
AF = mybir.ActivationFunctionType
ALU = mybir.AluOpType
BF = ml_dtypes.bfloat16


# ---------------------------------------------------------------- numpy prep
def _constants():
    E_all = np.zeros((128, 128), BF)
    F = np.zeros((128, 32), BF)
    for di in range(16):
        for l in range(L):
            F[di * 8 + l, di] = 1.0
            for j in range(4):
                E_all[32 * j + di, di * 8 + l] = 1.0
    return E_all, F


def _prep_core(x, weight, r):
    i0 = I_LOC * r
    xs = x[:, i0:i0 + I_LOC, :]                       # [B,144,8]
    ws = weight[:, i0:i0 + I_LOC, :, :]               # [10,144,8,16]
    x_il = np.ascontiguousarray(xs.transpose(1, 2, 0).reshape(I_LOC * L, B))
    xT = np.ascontiguousarray(
        x_il.reshape(NCHUNK, 128, B).transpose(1, 0, 2)[:, PERM]
        .reshape(128, NCHUNK * B)
    )
    w_il = ws.reshape(NN, I_LOC * L, O)               # [n,(il),o]
    WA = np.zeros((128, NN * NCHUNK, 32), BF)
    WA[:, :, :O] = w_il.reshape(NN, NCHUNK, 128, O).transpose(2, 0, 1, 3) \
        .reshape(128, NN * NCHUNK, O)
    WA = np.ascontiguousarray(WA.reshape(128, NN * NCHUNK * 32))
    WT = np.ascontiguousarray(w_il.transpose(0, 2, 1).reshape(NN * O, I_LOC * L))
    WT = np.ascontiguousarray(
        WT.reshape(NN, O, I_LOC * L).transpose(1, 0, 2)
        .reshape(O, NN * I_LOC * L).astype(BF)
    )
    return xT, xT.astype(BF), WA, WT


def _in_maps(x, weight):
    E_all, F = _constants()
    maps = []
    for r in range(N_CORES):
        xT, xTb, WA, WT = _prep_core(x, weight, r)
        maps.append({
            "xT": xT, "xTb": xTb, "WA": WA, "WT": WT,
            "EALL": E_all, "FMAT": F,
            "EYE16": np.eye(16, dtype=np.float32),
            "ONES16": np.ones((16, 1), np.float32),
            "ONES1x16": np.ones((1, 16), np.float32),
            "ONES1x128": np.ones((1, 128), np.float32),
        })
    return maps


# ---------------------------------------------------------------- bass build
def build_nc():
    nc = bacc.Bacc(
        "TRN2",
        target_bir_lowering=False,
        debug=False,
        enable_asserts=False,
        num_devices=N_CORES,
    )
    d_xT = nc.dram_tensor("xT", [128, NCHUNK * B], F32, kind="ExternalInput")
    d_xTb = nc.dram_tensor("xTb", [128, NCHUNK * B], BF16, kind="ExternalInput")
    d_WA = nc.dram_tensor("WA", [128, NN * NCHUNK * 32], BF16, kind="ExternalInput")
    d_WT = nc.dram_tensor("WT", [O, NN * I_LOC * L], BF16, kind="ExternalInput")
    d_E = nc.dram_tensor("EALL", [128, 128], BF16, kind="ExternalInput")
    d_F = nc.dram_tensor("FMAT", [128, 32], BF16, kind="ExternalInput")
    d_I16 = nc.dram_tensor("EYE16", [16, 16], F32, kind="ExternalInput")
    d_o16 = nc.dram_tensor("ONES16", [16, 1], F32, kind="ExternalInput")
    d_o1x = nc.dram_tensor("ONES1x16", [1, 16], F32, kind="ExternalInput")
    d_o1y = nc.dram_tensor("ONES1x128", [1, 128], F32, kind="ExternalInput")
    d_out = nc.dram_tensor("v_out", [NN * B, O], F32, kind="ExternalOutput")

    with tile.TileContext(nc) as tc:
        with (
            tc.tile_pool(name="persist", bufs=1) as pp,
            tc.tile_pool(name="work", bufs=10) as wp,
            tc.tile_pool(name="ps_s", bufs=1, space="PSUM") as ps_s,
            tc.tile_pool(name="ps_big", bufs=4, space="PSUM") as ps_big,
            tc.tile_pool(name="ps_delta", bufs=3, space="PSUM") as ps_delta,
            tc.tile_pool(name="dram", bufs=6, space="DRAM") as dp,
        ):
            # ---- persistent SBUF
            xT = pp.tile([128, NCHUNK * B], F32, tag="xT")
            xTb = pp.tile([128, NCHUNK * B], BF16, tag="xTb")
            WA = pp.tile([128, NN * NCHUNK * 32], BF16, tag="WA")
            WT = pp.tile([O, NN * I_LOC * L], BF16, tag="WT")
            EALL = pp.tile([128, 128], BF16, tag="EALL")
            FMAT = pp.tile([128, 32], BF16, tag="FMAT")
            EYE16 = pp.tile([16, 16], F32, tag="EYE16")
            ONES16 = pp.tile([16, 1], F32, tag="ONES16")
            ONES1x16 = pp.tile([1, 16], F32, tag="ONES1x16")
            ONES1x128 = pp.tile([1, 128], F32, tag="ONES1x128")
            bb = pp.tile([128, NTILE * B], F32, tag="bb")
            expb = pp.tile([128, NTILE * B], F32, tag="expb")
            csb = pp.tile([128, NTILE * B], BF16, tag="csb")
            den = pp.tile([128, NTILE], F32, tag="den")
            denr = pp.tile([128, NTILE], F32, tag="denr")
            s_stage4 = pp.tile([128, 3 * B], F32, tag="s_stage4")
            ssum = pp.tile([O, NN * B], F32, tag="ssum")
            ssb = pp.tile([O, NN * B], BF16, tag="ssb")
            sq_scr = pp.tile([O, NN * B], F32, tag="sq_scr")
            vsb = pp.tile([O, NN * B], F32, tag="vsb")
            vout = pp.tile([128, 20 * O], F32, tag="vout")
            q16 = pp.tile([O, 1], F32, tag="q16")
            sc_r = pp.tile([1, 1], F32, tag="sc_r")
            sc_d = pp.tile([1, 1], F32, tag="sc_d")
            sc_dr = pp.tile([1, 1], F32, tag="sc_dr")
            sc_f = pp.tile([1, 1], F32, tag="sc_f")
            f16 = pp.tile([O, 1], F32, tag="f16")
            f128 = pp.tile([128, 1], F32, tag="f128")

            # ---- load inputs
            nc.sync.dma_start(xTb[:], d_xTb.ap())
            for n_ in range(NN):
                w0, w1 = n_ * NCHUNK * 32, (n_ + 1) * NCHUNK * 32
                nc.sync.dma_start(WA[:, w0:w1], d_WA.ap()[:, w0:w1])
            nc.sync.dma_start(xT[:], d_xT.ap())
            nc.sync.dma_start(WT[:], d_WT.ap())
            nc.sync.dma_start(EALL[:], d_E.ap())
            nc.sync.dma_start(FMAT[:], d_F.ap())
            nc.sync.dma_start(EYE16[:], d_I16.ap())
            nc.sync.dma_start(ONES16[:], d_o16.ap())
            nc.sync.dma_start(ONES1x16[:], d_o1x.ap())
            nc.sync.dma_start(ONES1x128[:], d_o1y.ap())
            nc.gpsimd.memset(bb[:], 0.0)

            cc_in = [
                dp.tile([O, NN * B], F32, tag=f"cc_in{k}", name=f"cc_in{k}")
                for k in range(3)
            ]
            cc_out = [
                dp.tile([O, NN * B], F32, tag=f"cc_out{k}", name=f"cc_out{k}")
                for k in range(3)
            ]

            def ck_pairs():
                return [(0, 2), (2, 4), (4, 6), (6, 8), (8, 9)]

            def s_pass(k, rhs_of, n0=None):
                """s-matmuls packed 4 classes per PSUM tile at col strips
                (stationary padded to M=32 so strips are fully written);
                drain via SBUF staging, then partition-shifting DMAs into
                the collective DRAM buffer."""
                packs = range(0, NN, 4) if n0 is None else [n0]
                for n0 in packs:
                    nhi = min(n0 + 4, NN)
                    pk = n0 // 4
                    s4 = ps_s.tile([128, B], F32, tag="s_acc", name=f"s4_{k}_{n0}")
                    for n in range(n0, nhi):
                        jn = n - n0
                        for ck in range(NCHUNK):
                            g = n * NCHUNK + ck
                            nc.tensor.matmul(
                                s4[32 * jn:32 * jn + 32, :],
                                lhsT=WA[:, g * 32:(g + 1) * 32],
                                rhs=rhs_of(n, ck),
                                start=(ck == 0),
                                stop=(ck == NCHUNK - 1),
                                tile_position=(0, 32 * jn),
                            )
                    p_hi = 32 * (nhi - n0)
                    nc.scalar.activation(
                        s_stage4[:p_hi, pk * B:(pk + 1) * B], s4[:p_hi, :], AF.Copy
                    )
                    for n in range(n0, nhi):
                        jn = n - n0
                        nc.sync.dma_start(
                            cc_in[k][:, n * B:(n + 1) * B],
                            s_stage4[32 * jn:32 * jn + 16, pk * B:(pk + 1) * B],
                        )

            def allreduce(k, alpha, last):
                nc.gpsimd.collective_compute(
                    "AllReduce",
                    ALU.add,
                    replica_groups=[list(range(N_CORES))],
                    ins=[cc_in[k].opt()],
                    outs=[cc_out[k].opt()],
                )
                nc.sync.dma_start(ssum[:], cc_out[k][:])
                if not last:
                    for n_ in range(NN):
                        nc.scalar.activation(
                            ssb[:, n_ * B:(n_ + 1) * B],
                            ssum[:, n_ * B:(n_ + 1) * B], AF.Copy,
                        )
                # squash scalar chain -> factor (applied later at bb-add / v)
                nc.scalar.activation(sq_scr[:], ssum[:], AF.Square, accum_out=q16[:])
                n2_ps = ps_delta.tile([1, 1], F32, tag="delta", name=f"n2_{k}")
                nc.tensor.matmul(n2_ps[:], lhsT=ONES16[:], rhs=q16[:])
                a2 = float(alpha * alpha)
                nc.scalar.activation(sc_r[:], n2_ps[:], AF.Sqrt, scale=a2)
                nc.scalar.activation(sc_d[:], n2_ps[:], AF.Copy, bias=1.0, scale=a2)
                nc.vector.reciprocal(sc_dr[:], sc_d[:])
                nc.vector.scalar_tensor_tensor(
                    out=sc_f[:], in0=sc_r[:], scalar=float(alpha), in1=sc_dr[:],
                    op0=ALU.mult, op1=ALU.mult,
                )
                if last:
                    f16_ps = ps_delta.tile([O, 1], F32, tag="delta", name=f"f16_{k}")
                    nc.tensor.matmul(f16_ps[:], lhsT=ONES1x16[:], rhs=sc_f[:])
                    nc.vector.tensor_copy(f16[:], f16_ps[:])
                    for n in range(NN):
                        nc.scalar.activation(
                            vsb[:, n * B:(n + 1) * B], ssum[:, n * B:(n + 1) * B],
                            AF.Copy, scale=f16[:],
                        )
                else:
                    f128_ps = ps_delta.tile([128, 1], F32, tag="delta", name=f"f128_{k}")
                    nc.tensor.matmul(f128_ps[:], lhsT=ONES1x128[:], rhs=sc_f[:])
                    nc.vector.tensor_copy(f128[:], f128_ps[:])

            # ================= phase 1: uniform-c s1 =================
            with nc.named_scope("phase_s1"):
                s_pass(0, lambda n, ck: xTb[:, COL[ck] * B:(COL[ck] + 1) * B])
            with nc.named_scope("ar1"):
                allreduce(0, 1.0 / B, last=False)

            # ================= routing phases 2,3 =================
            for it in (1, 2):
                with nc.named_scope(f"bbupd{it}"):
                    # G'-mms on un-squashed s_sum; factor folded into bb-add
                    deltas = {}

                    def fmm(g, rhs_ap, it=it):
                        t, j = g // 4, g % 4
                        if t not in deltas:
                            deltas[t] = [
                                ps_delta.tile([128, B], F32, tag="delta",
                                              name=f"delta_{it}_{t}"),
                                0,
                            ]
                        ent = deltas[t]
                        nc.tensor.matmul(
                            ent[0][32 * j:32 * j + 32, :],
                            lhsT=FMAT[:], rhs=rhs_ap,
                            tile_position=(0, 32 * j),
                        )
                        ent[1] += 1
                        full = 2 if t == NTILE - 1 else 4
                        if ent[1] == full:
                            p_hi = 32 * full
                            nc.vector.scalar_tensor_tensor(
                                out=bb[:p_hi, t * B:(t + 1) * B],
                                in0=ent[0][:p_hi, :],
                                scalar=f128[:p_hi, 0:1],
                                in1=bb[:p_hi, t * B:(t + 1) * B],
                                op0=ALU.mult, op1=ALU.add,
                            )
                            del deltas[t]

                    xT9 = xT[:].rearrange("p (c b) -> p c b", c=NCHUNK)
                    for n in range(NN):
                        tmps = {}
                        for (c0, c1) in ck_pairs():
                            w = (c1 - c0) * B
                            G = ps_big.tile([128, 2 * B], F32, tag="big",
                                            name=f"G_{it}_{n}_{c0}")
                            for ck in range(c0, c1):
                                h = ck - c0
                                nc.tensor.matmul(
                                    G[:, h * B:(h + 1) * B],
                                    lhsT=WT[:, n * 1152 + ck * 128: n * 1152 + (ck + 1) * 128],
                                    rhs=ssb[:, n * B:(n + 1) * B],
                                )
                            tmp = wp.tile([128, 2 * B], BF16, tag="tmp")
                            if c1 - c0 == 2:
                                in0 = xT9[:, COL[c0]:COL[c0] + 3:2, :]
                                nc.vector.tensor_mul(
                                    tmp[:].rearrange("p (c b) -> p c b", c=2),
                                    in0,
                                    G[:].rearrange("p (c b) -> p c b", c=2),
                                )
                            else:
                                nc.vector.tensor_mul(
                                    tmp[:, :w], xT[:, COL[c0] * B:(COL[c0] + 1) * B],
                                    G[:, :w],
                                )
                            for ck in range(c0, c1):
                                tmps[ck] = (tmp, ck - c0)
                        for ck in range(NCHUNK):
                            tmp, h = tmps[ck]
                            fmm(n * NCHUNK + ck, tmp[:, h * B:(h + 1) * B])
                with nc.named_scope(f"softmax{it}"):
                    for t in range(NTILE):
                        nc.scalar.activation(
                            expb[:, t * B:(t + 1) * B], bb[:, t * B:(t + 1) * B],
                            AF.Exp, accum_out=den[:, t:t + 1],
                        )
                        nc.vector.reciprocal(denr[:, t:t + 1], den[:, t:t + 1])
                    for t in range(NTILE):
                        nc.scalar.activation(
                            csb[:, t * B:(t + 1) * B], expb[:, t * B:(t + 1) * B],
                            AF.Copy, scale=denr[:, t:t + 1],
                        )
                with nc.named_scope(f"schain{it}"):
                    xc_of = {}

                    def emit_exc(n, it=it):
                        for pi, pair in enumerate(PAIRS):
                            w = len(pair) * B
                            g0 = n * NCHUNK + pair[0]
                            t, j = g0 // 4, g0 % 4
                            ct = ps_big.tile([128, 2 * B], F32, tag="big",
                                             name=f"ct_{it}_{n}_{pi}")
                            nc.tensor.matmul(
                                ct[:, :w],
                                lhsT=EALL[32 * j:32 * j + 16, :],
                                rhs=csb[32 * j:32 * j + 16, t * B:t * B + w],
                                tile_position=(32 * j, 0),
                            )
                            xc = wp.tile([128, 2 * B], BF16, tag="xc")
                            nc.vector.tensor_mul(
                                xc[:, :w],
                                xT[:, 2 * pi * B:2 * pi * B + w], ct[:, :w]
                            )
                            for h, ck in enumerate(pair):
                                xc_of[(n, ck)] = (xc, h)

                    def rhs_of(n, ck):
                        xc, h = xc_of[(n, ck)]
                        return xc[:, h * B:(h + 1) * B]

                    for n0 in range(0, NN, 4):
                        nhi = min(n0 + 4, NN)
                        pk = n0 // 4
                        s4 = ps_s.tile([128, B], F32, tag="s_acc",
                                       name=f"s4i_{it}_{n0}")
                        for n in range(n0, nhi):
                            emit_exc(n)
                            jn = n - n0
                            for ck in range(NCHUNK):
                                g = n * NCHUNK + ck
                                nc.tensor.matmul(
                                    s4[32 * jn:32 * jn + 32, :],
                                    lhsT=WA[:, g * 32:(g + 1) * 32],
                                    rhs=rhs_of(n, ck),
                                    start=(ck == 0),
                                    stop=(ck == NCHUNK - 1),
                                    tile_position=(0, 32 * jn),
                                )
                        p_hi = 32 * (nhi - n0)
                        nc.scalar.activation(
                            s_stage4[:p_hi, pk * B:(pk + 1) * B], s4[:p_hi, :],
                            AF.Copy,
                        )
                        for n in range(n0, nhi):
                            jn = n - n0
                            nc.sync.dma_start(
                                cc_in[it][:, n * B:(n + 1) * B],
                                s_stage4[32 * jn:32 * jn + 16, pk * B:(pk + 1) * B],
                            )
                with nc.named_scope(f"ar{it + 1}"):
                    allreduce(it, 1.0, last=(it == 2))

            # ================= output =================
            with nc.named_scope("out"):
                for k in range(20):
                    vt = ps_delta.tile([128, O], F32, tag="delta", name=f"vt_{k}")
                    nc.tensor.transpose(
                        vt[:], vsb[:, k * 128:(k + 1) * 128], EYE16[:]
                    )
                    nc.scalar.activation(vout[:, k * O:(k + 1) * O], vt[:], AF.Copy)
                nc.sync.dma_start(
                    d_out.ap().rearrange("(k p) o -> p k o", p=128),
                    vout[:].rearrange("p (k o) -> p k o", k=20),
                )
    nc.compile()
    return nc


_NC = None


def _get_nc():
    global _NC
    if _NC is None:
        _NC = build_nc()
    return _NC


def run_spmd(x, weight, trace=False, **kw):
    nc = _get_nc()
    res = bass_utils.run_bass_kernel_spmd(
        nc, _in_maps(np.asarray(x), np.asarray(weight)),
        core_ids=list(range(N_CORES)), trace=trace, **kw,
    )
    return res


def kernel(x, weight):
    res = run_spmd(x, weight, trace=False)
    v = res.results[0]["v_out"]                    # [2560, 16]
    return v.reshape(NN, B, 1, 1, O).astype(np.float32)


# revision 26
# speedup vs baseline: 1.3464x; 1.0093x over previous
"""Trainium2 Bass kernel for nn_DigitCapsLayer (dynamic routing capsule layer).

Strategy: shard the 1152-wide input-capsule axis across 8 cores (144 each).
Priors P = x@W are never materialized; each routing iteration runs as bf16
matmuls on the tensor engine (fp32 PSUM accumulate):
  - s_n = sum_il (x^T * c_bcast) W_n      (xc elementwise on DVE, PE contract)
  - c broadcast over l via constant selector matmul E (exact 0/1 in bf16)
  - bb update delta = F^T (x^T * (W_n^T @ s_sum)), squash factor folded into
    the bb accumulate (linear), so G-matmuls start right after the AllReduce
Softmax over batch is along the free dim per (n,i) row; (n,i) rows are packed
4-per-128-partition-tile at 32-aligned strips to satisfy PE tile_position
rules. s partials are AllReduced (160KB fp32) once per routing iteration.

Self-contained: hardcodes shapes from the problem spec.
"""
import os
import sys
import types

import numpy as np

sys.path.insert(0, "/root/.axon_site")
try:  # NTFF profile hook shim (timing only; harmless if unavailable)
    import antenv.axon_hooks  # noqa: F401
except ImportError:
    try:
        from trn_agent_boot import trn_boot as _tb

        _m = types.ModuleType("antenv.axon_hooks")
        _hook = _tb._ntff_profile_via_ctypes("/opt/axon/libaxon_pjrt.so")
        _m.get_axon_ntff_profile_hook = lambda: _hook
        sys.modules["antenv.axon_hooks"] = _m
    except Exception:
        pass

import ml_dtypes

import concourse.bacc as bacc
import concourse.mybir as mybir
import concourse.tile as tile
from concourse import bass_utils

N_CORES = 8
NN = 10       # output capsule classes
B = 256       # batch
I_LOC = 144   # input capsules per core
L = 8         # in capsule dim
O = 16        # out capsule dim
NCHUNK = 9    # 128-row (i,l) chunks per core
NGROUP = NN * NCHUNK          # 90 groups of 16 i's
NTILE = (NGROUP + 3) // 4     # 23 packed bb tiles (4 strips each)
PERM = [0, 4, 1, 5, 2, 6, 3, 7, 8]            # chunk order in xT columns
COL = {ck: i for i, ck in enumerate(PERM)}    # chunk -> xT column block
PAIRS = [(0, 4), (1, 5), (2, 6), (3, 7), (8,)]
F32 = mybir.dt.float32
BF16 = mybir.dt.bfloat16
AF = mybir.ActivationFunctionType
ALU = mybir.AluOpType
BF = ml_dtypes.bfloat16


# ---------------------------------------------------------------- numpy prep
def _constants():
    E_all = np.zeros((128, 128), BF)
    F = np.zeros((128, 32), BF)
    for di in range(16):
        for l in range(L):
            F[di * 8 + l, di] = 1.0
            for j in range(4):
                E_all[32 * j + di, di * 8 + l] = 1.0
    return E_all, F


def _prep_core(x, weight, r):
    i0 = I_LOC * r
    xs = x[:, i0:i0 + I_LOC, :]                       # [B,144,8]
    ws = weight[:, i0:i0 + I_LOC, :, :]               # [10,144,8,16]
    x_il = np.ascontiguousarray(xs.transpose(1, 2, 0).reshape(I_LOC * L, B))
    xT = np.ascontiguousarray(
        x_il.reshape(NCHUNK, 128, B).transpose(1, 0, 2)[:, PERM]
        .reshape(128, NCHUNK * B)
    )
    w_il = ws.reshape(NN, I_LOC * L, O)               # [n,(il),o]
    WA = np.zeros((128, NN * NCHUNK, 32), BF)
    WA[:, :, :O] = w_il.reshape(NN, NCHUNK, 128, O).transpose(2, 0, 1, 3) \
        .reshape(128, NN * NCHUNK, O)
    WA = np.ascontiguousarray(WA.reshape(128, NN * NCHUNK * 32))
    WT = np.ascontiguousarray(w_il.transpose(0, 2, 1).reshape(NN * O, I_LOC * L))
    WT = np.ascontiguousarray(
        WT.reshape(NN, O, I_LOC * L).transpose(1, 0, 2)
        .reshape(O, NN * I_LOC * L).astype(BF)
    )
    return xT, xT.astype(BF), WA, WT


def _in_maps(x, weight):
    E_all, F = _constants()
    maps = []
    for r in range(N_CORES):
        xT, xTb, WA, WT = _prep_core(x, weight, r)
        maps.append({
            "xT": xT, "xTb": xTb, "WA": WA, "WT": WT,
            "EALL": E_all, "FMAT": F,
            "EYE16": np.eye(16, dtype=np.float32),
            "ONES16": np.ones((16, 1), np.float32),
            "ONES1x16": np.ones((1, 16), np.float32),
            "ONES1x128": np.ones((1, 128), np.float32),
        })
    return maps


# ---------------------------------------------------------------- bass build
def build_nc():
    nc = bacc.Bacc(
        "TRN2",
        target_bir_lowering=False,
        debug=False,
        enable_asserts=False,
        num_devices=N_CORES,
    )
    d_xT = nc.dram_tensor("xT", [128, NCHUNK * B], F32, kind="ExternalInput")
    d_xTb = nc.dram_tensor("xTb", [128, NCHUNK * B], BF16, kind="ExternalInput")
    d_WA = nc.dram_tensor("WA", [128, NN * NCHUNK * 32], BF16, kind="ExternalInput")
    d_WT = nc.dram_tensor("WT", [O, NN * I_LOC * L], BF16, kind="ExternalInput")
    d_E = nc.dram_tensor("EALL", [128, 128], BF16, kind="ExternalInput")
    d_F = nc.dram_tensor("FMAT", [128, 32], BF16, kind="ExternalInput")
    d_I16 = nc.dram_tensor("EYE16", [16, 16], F32, kind="ExternalInput")
    d_o16 = nc.dram_tensor("ONES16", [16, 1], F32, kind="ExternalInput")
    d_o1x = nc.dram_tensor("ONES1x16", [1, 16], F32, kind="ExternalInput")
    d_o1y = nc.dram_tensor("ONES1x128", [1, 128], F32, kind="ExternalInput")
    d_out = nc.dram_tensor("v_out", [NN * B, O], F32, kind="ExternalOutput")

    with tile.TileContext(nc) as tc:
        with (
            tc.tile_pool(name="persist", bufs=1) as pp,
            tc.tile_pool(name="work", bufs=10) as wp,
            tc.tile_pool(name="ps_s", bufs=2, space="PSUM") as ps_s,
            tc.tile_pool(name="ps_big", bufs=4, space="PSUM") as ps_big,
            tc.tile_pool(name="ps_delta", bufs=2, space="PSUM") as ps_delta,
            tc.tile_pool(name="dram", bufs=6, space="DRAM") as dp,
        ):
            # ---- persistent SBUF
            xT = pp.tile([128, NCHUNK * B], F32, tag="xT")
            xTb = pp.tile([128, NCHUNK * B], BF16, tag="xTb")
            WA = pp.tile([128, NN * NCHUNK * 32], BF16, tag="WA")
            WT = pp.tile([O, NN * I_LOC * L], BF16, tag="WT")
            EALL = pp.tile([128, 128], BF16, tag="EALL")
            FMAT = pp.tile([128, 32], BF16, tag="FMAT")
            EYE16 = pp.tile([16, 16], F32, tag="EYE16")
            ONES16 = pp.tile([16, 1], F32, tag="ONES16")
            ONES1x16 = pp.tile([1, 16], F32, tag="ONES1x16")
            ONES1x128 = pp.tile([1, 128], F32, tag="ONES1x128")
            bb = pp.tile([128, NTILE * B], F32, tag="bb")
            expb = pp.tile([128, NTILE * B], F32, tag="expb")
            csb = pp.tile([128, NTILE * B], BF16, tag="csb")
            den = pp.tile([128, NTILE], F32, tag="den")
            denr = pp.tile([128, NTILE], F32, tag="denr")
            s_stage4 = pp.tile([128, 3 * B], F32, tag="s_stage4")
            ssum = pp.tile([O, NN * B], F32, tag="ssum")
            ssb = pp.tile([O, NN * B], BF16, tag="ssb")
            sq_scr = pp.tile([O, NN * B], F32, tag="sq_scr")
            vsb = pp.tile([O, NN * B], F32, tag="vsb")
            vout = pp.tile([128, 20 * O], F32, tag="vout")
            q16 = pp.tile([O, 1], F32, tag="q16")
            sc_r = pp.tile([1, 1], F32, tag="sc_r")
            sc_d = pp.tile([1, 1], F32, tag="sc_d")
            sc_dr = pp.tile([1, 1], F32, tag="sc_dr")
            sc_f = pp.tile([1, 1], F32, tag="sc_f")
            f16 = pp.tile([O, 1], F32, tag="f16")
            f128 = pp.tile([128, 1], F32, tag="f128")

            # ---- load inputs
            nc.sync.dma_start(xTb[:], d_xTb.ap())
            for n_ in range(NN):
                w0, w1 = n_ * NCHUNK * 32, (n_ + 1) * NCHUNK * 32
                nc.sync.dma_start(WA[:, w0:w1], d_WA.ap()[:, w0:w1])
            nc.sync.dma_start(xT[:], d_xT.ap())
            nc.sync.dma_start(WT[:], d_WT.ap())
            nc.sync.dma_start(EALL[:], d_E.ap())
            nc.sync.dma_start(FMAT[:], d_F.ap())
            nc.sync.dma_start(EYE16[:], d_I16.ap())
            nc.sync.dma_start(ONES16[:], d_o16.ap())
            nc.sync.dma_start(ONES1x16[:], d_o1x.ap())
            nc.sync.dma_start(ONES1x128[:], d_o1y.ap())
            nc.gpsimd.memset(bb[:], 0.0)

            cc_in = [
                dp.tile([O, NN * B], F32, tag=f"cc_in{k}", name=f"cc_in{k}")
                for k in range(3)
            ]
            cc_out = [
                dp.tile([O, NN * B], F32, tag=f"cc_out{k}", name=f"cc_out{k}")
                for k in range(3)
            ]

            def ck_pairs():
                return [(0, 2), (2, 4), (4, 6), (6, 8), (8, 9)]

            def s_pass(k, rhs_of, n0=None):
                """s-matmuls packed 4 classes per PSUM tile at col strips
                (stationary padded to M=32 so strips are fully written);
                drain via SBUF staging, then partition-shifting DMAs into
                the collective DRAM buffer."""
                packs = range(0, NN, 4) if n0 is None else [n0]
                for n0 in packs:
                    nhi = min(n0 + 4, NN)
                    pk = n0 // 4
                    s4 = ps_s.tile([128, B], F32, tag="s_acc", name=f"s4_{k}_{n0}")
                    for n in range(n0, nhi):
                        jn = n - n0
                        for ck in range(NCHUNK):
                            g = n * NCHUNK + ck
                            nc.tensor.matmul(
                                s4[32 * jn:32 * jn + 32, :],
                                lhsT=WA[:, g * 32:(g + 1) * 32],
                                rhs=rhs_of(n, ck),
                                start=(ck == 0),
                                stop=(ck == NCHUNK - 1),
                                tile_position=(0, 32 * jn),
                            )
                    p_hi = 32 * (nhi - n0)
                    nc.scalar.activation(
                        s_stage4[:p_hi, pk * B:(pk + 1) * B], s4[:p_hi, :], AF.Copy
                    )
                    for n in range(n0, nhi):
                        jn = n - n0
                        nc.sync.dma_start(
                            cc_in[k][:, n * B:(n + 1) * B],
                            s_stage4[32 * jn:32 * jn + 16, pk * B:(pk + 1) * B],
                        )

            def allreduce(k, alpha, last):
                nc.gpsimd.collective_compute(
                    "AllReduce",
                    ALU.add,
                    replica_groups=[list(range(N_CORES))],
                    ins=[cc_in[k].opt()],
                    outs=[cc_out[k].opt()],
                )
                nc.sync.dma_start(ssum[:], cc_out[k][:])
                if not last:
                    for n_ in range(NN):
                        nc.scalar.activation(
                            ssb[:, n_ * B:(n_ + 1) * B],
                            ssum[:, n_ * B:(n_ + 1) * B], AF.Copy,
                        )
                # squash scalar chain -> factor (applied later at bb-add / v)
                nc.scalar.activation(sq_scr[:], ssum[:], AF.Square, accum_out=q16[:])
                n2_ps = ps_delta.tile([1, 1], F32, tag="delta", name=f"n2_{k}")
                nc.tensor.matmul(n2_ps[:], lhsT=ONES16[:], rhs=q16[:])
                a2 = float(alpha * alpha)
                nc.scalar.activation(sc_r[:], n2_ps[:], AF.Sqrt, scale=a2)
                nc.scalar.activation(sc_d[:], n2_ps[:], AF.Copy, bias=1.0, scale=a2)
                nc.vector.reciprocal(sc_dr[:], sc_d[:])
                nc.vector.scalar_tensor_tensor(
                    out=sc_f[:], in0=sc_r[:], scalar=float(alpha), in1=sc_dr[:],
                    op0=ALU.mult, op1=ALU.mult,
                )
                if last:
                    f16_ps = ps_delta.tile([O, 1], F32, tag="delta", name=f"f16_{k}")
                    nc.tensor.matmul(f16_ps[:], lhsT=ONES1x16[:], rhs=sc_f[:])
                    nc.vector.tensor_copy(f16[:], f16_ps[:])
                    for n in range(NN):
                        nc.scalar.activation(
                            vsb[:, n * B:(n + 1) * B], ssum[:, n * B:(n + 1) * B],
                            AF.Copy, scale=f16[:],
                        )
                else:
                    f128_ps = ps_delta.tile([128, 1], F32, tag="delta", name=f"f128_{k}")
                    nc.tensor.matmul(f128_ps[:], lhsT=ONES1x128[:], rhs=sc_f[:])
                    nc.vector.tensor_copy(f128[:], f128_ps[:])

            # ================= phase 1: uniform-c s1 =================
            with nc.named_scope("phase_s1"):
                s_pass(0, lambda n, ck: xTb[:, COL[ck] * B:(COL[ck] + 1) * B])
            with nc.named_scope("ar1"):
                allreduce(0, 1.0 / B, last=False)

            # ================= routing phases 2,3 =================
            for it in (1, 2):
                with nc.named_scope(f"bbupd{it}"):
                    # G'-mms on un-squashed s_sum; factor folded into bb-add
                    deltas = {}

                    def fmm(g, rhs_ap, it=it):
                        t, j = g // 4, g % 4
                        if t not in deltas:
                            deltas[t] = [
                                ps_delta.tile([128, B], F32, tag="delta",
                                              name=f"delta_{it}_{t}"),
                                0,
                            ]
                        ent = deltas[t]
                        nc.tensor.matmul(
                            ent[0][32 * j:32 * j + 32, :],
                            lhsT=FMAT[:], rhs=rhs_ap,
                            tile_position=(0, 32 * j),
                        )
                        ent[1] += 1
                        full = 2 if t == NTILE - 1 else 4
                        if ent[1] == full:
                            p_hi = 32 * full
                            nc.vector.scalar_tensor_tensor(
                                out=bb[:p_hi, t * B:(t + 1) * B],
                                in0=ent[0][:p_hi, :],
                                scalar=f128[:p_hi, 0:1],
                                in1=bb[:p_hi, t * B:(t + 1) * B],
                                op0=ALU.mult, op1=ALU.add,
                            )
                            del deltas[t]

                    xT9 = xT[:].rearrange("p (c b) -> p c b", c=NCHUNK)
                    for n in range(NN):
                        tmps = {}
                        for (c0, c1) in ck_pairs():
                            w = (c1 - c0) * B
                            G = ps_big.tile([128, 2 * B], F32, tag="big",
                                            name=f"G_{it}_{n}_{c0}")
                            for ck in range(c0, c1):
                                h = ck - c0
                                nc.tensor.matmul(
                                    G[:, h * B:(h + 1) * B],
                                    lhsT=WT[:, n * 1152 + ck * 128: n * 1152 + (ck + 1) * 128],
                                    rhs=ssb[:, n * B:(n + 1) * B],
                                )
                            tmp = wp.tile([128, 2 * B], BF16, tag="tmp")
                            if c1 - c0 == 2:
                                in0 = xT9[:, COL[c0]:COL[c0] + 3:2, :]
                                nc.vector.tensor_mul(
                                    tmp[:].rearrange("p (c b) -> p c b", c=2),
                                    in0,
                                    G[:].rearrange("p (c b) -> p c b", c=2),
                                )
                            else:
                                nc.vector.tensor_mul(
                                    tmp[:, :w], xT[:, COL[c0] * B:(COL[c0] + 1) * B],
                                    G[:, :w],
                                )
                            for ck in range(c0, c1):
                                tmps[ck] = (tmp, ck - c0)
                        for ck in range(NCHUNK):
                            tmp, h = tmps[ck]
                            fmm(n * NCHUNK + ck, tmp[:, h * B:(h + 1) * B])
                with nc.named_scope(f"softmax{it}"):
                    for t in range(NTILE):
                        nc.scalar.activation(
                            expb[:, t * B:(t + 1) * B], bb[:, t * B:(t + 1) * B],
                            AF.Exp, accum_out=den[:, t:t + 1],
                        )
                        nc.vector.reciprocal(denr[:, t:t + 1], den[:, t:t + 1])
                    for t in range(NTILE):
                        nc.scalar.activation(
                            csb[:, t * B:(t + 1) * B], expb[:, t * B:(t + 1) * B],
                            AF.Copy, scale=denr[:, t:t + 1],
                        )
                with nc.named_scope(f"schain{it}"):
                    xc_of = {}

                    def emit_exc(n, it=it):
                        for pi, pair in enumerate(PAIRS):
                            w = len(pair) * B
                            g0 = n * NCHUNK + pair[0]
                            t, j = g0 // 4, g0 % 4
                            ct = ps_big.tile([128, 2 * B], F32, tag="big",
                                             name=f"ct_{it}_{n}_{pi}")
                            nc.tensor.matmul(
                                ct[:, :w],
                                lhsT=EALL[32 * j:32 * j + 16, :],
                                rhs=csb[32 * j:32 * j + 16, t * B:t * B + w],
                                tile_position=(32 * j, 0),
                            )
                            xc = wp.tile([128, 2 * B], BF16, tag="xc")
                            nc.vector.tensor_mul(
                                xc[:, :w],
                                xT[:, 2 * pi * B:2 * pi * B + w], ct[:, :w]
                            )
                            for h, ck in enumerate(pair):
                                xc_of[(n, ck)] = (xc, h)

                    def rhs_of(n, ck):
                        xc, h = xc_of[(n, ck)]
                        return xc[:, h * B:(h + 1) * B]

                    for n0 in range(0, NN, 4):
                        nhi = min(n0 + 4, NN)
                        pk = n0 // 4
                        s4 = ps_s.tile([128, B], F32, tag="s_acc",
                                       name=f"s4i_{it}_{n0}")
                        for n in range(n0, nhi):
                            emit_exc(n)
                            jn = n - n0
                            for ck in range(NCHUNK):
                                g = n * NCHUNK + ck
                                nc.tensor.matmul(
                                    s4[32 * jn:32 * jn + 32, :],
                                    lhsT=WA[:, g * 32:(g + 1) * 32],
                                    rhs=rhs_of(n, ck),
                                    start=(ck == 0),
                                    stop=(ck == NCHUNK - 1),
                                    tile_position=(0, 32 * jn),
                                )
                        p_hi = 32 * (nhi - n0)
                        nc.scalar.activation(
                            s_stage4[:p_hi, pk * B:(pk + 1) * B], s4[:p_hi, :],
                            AF.Copy,
                        )
                        for n in range(n0, nhi):
                            jn = n - n0
                            nc.sync.dma_start(
                                cc_in[it][:, n * B:(n + 1) * B],
                                s_stage4[32 * jn:32 * jn + 16, pk * B:(pk + 1) * B],
                            )
                with nc.named_scope(f"ar{it + 1}"):
                    allreduce(it, 1.0, last=(it == 2))

            # ================= output =================
            with nc.named_scope("out"):
                for k in range(20):
                    vt = ps_delta.tile([128, O], F32, tag="delta", name=f"vt_{k}")
                    nc.tensor.transpose(
                        vt[:], vsb[:, k * 128:(k + 1) * 128], EYE16[:]
                    )
                    nc.scalar.activation(vout[:, k * O:(k + 1) * O], vt[:], AF.Copy)
                nc.sync.dma_start(
                    d_out.ap().rearrange("(k p) o -> p k o", p=128),
                    vout[:].rearrange("p (k o) -> p k o", k=20),
                )
    nc.compile()
    return nc


_NC = None


def _get_nc():
    global _NC
    if _NC is None:
        _NC = build_nc()
    return _NC


def run_spmd(x, weight, trace=False, **kw):
    nc = _get_nc()
    res = bass_utils.run_bass_kernel_spmd(
        nc, _in_maps(np.asarray(x), np.asarray(weight)),
        core_ids=list(range(N_CORES)), trace=trace, **kw,
    )
    return res


def kernel(x, weight):
    res = run_spmd(x, weight, trace=False)
    v = res.results[0]["v_out"]                    # [2560, 16]
    return v.reshape(NN, B, 1, 1, O).astype(np.float32)
